# revision 47
# baseline (speedup 1.0000x reference)
"""BoundaryLoss kernel for 8 Trainium2 NeuronCores.

Math (equivalent to the reference):
  boundary(i,j) = [L(i,j+1) != L(i,j-1)]_edge OR [L(i+1,j) != L(i-1,j)]_edge
    (the union of class-1/class-2 indicator boundaries equals "any label
     change" because any differing pair in {0,1,2} differs in membership
     of class 1 or class 2; |gx|+|gy| > 0.1 iff either diff is nonzero)
  ce = logsumexp_c(x) - x[label]        (max-free: |x| <= ~6 so exp is safe)
  loss = sum(ce * boundary) / (sum(boundary) + 1e-8)

Sharding: pure data parallel, 4 images per core.  Each core writes
per-column partial sums of (boundary, ce*boundary); the host sums the
8 * [1, 1024] partials in float64 and does the final division.

Design notes (what made this fast; measured on HW traces):
  - ALL loads ride the pipelined HWDGE queue as plain dtypes.  The SWDGE
    (gpsimd) cast-DMA queue is serial (~6.5us per dispatch) and GpSimd
    *compute* locks the vector engine out of its 2-port perf modes
    (measured 2-3x slowdowns on concurrently-running DVE ops), so
    neither is used.  Label i32 -> bf16 casts run on the scalar engine.
  - Seam rows (partition-crossing row neighbours for the vertical
    gradient) come from small host-prepared halo tensors; partition-
    shifted on-device copies degrade to per-partition descriptors that
    pile up on one DMA engine.
  - x[label] is selected with copy_predicated overwrites (x0, then x1
    where L != 0 using the label bits as the predicate, then x2 where
    Relu(L-1) != 0) instead of exp-domain masked products + a second
    Ln: one activation pass and several DVE passes cheaper.
  - per-pixel boundary / ce*boundary sums reduce on the tensor engine
    (matmul with a ones vector), which is otherwise idle; work is
    balanced so scalar ~= vector ~= DMA ~= 120us per core.
"""

import numpy as np

B, C, H, W = 32, 3, 768, 768
NCORES = 8
BLOC = B // NCORES  # images per core
P = 128
TPB = H // P        # rows per partition (6)
NH = 2              # chunks (halves) per image
RPC = TPB // NH     # rows per chunk (3)
CHW = RPC * W       # columns per chunk (2304)

_CACHE = {}


def _build(label_words):
    """Build + compile the Bass module. label_words = int32 words per label
    element (2 for int64 inputs, 1 for int32)."""
    import concourse.bacc as bacc
    import concourse.tile as tile
    import concourse.mybir as mybir
    from bass_rust import add_dep_helper as _add_dep

    fp32 = mybir.dt.float32
    bf16 = mybir.dt.bfloat16
    i32 = mybir.dt.int32
    Alu = mybir.AluOpType
    Act = mybir.ActivationFunctionType

    nc = bacc.Bacc(
        "TRN2",
        target_bir_lowering=False,
        debug=False,
        enable_asserts=False,
        num_devices=NCORES,
    )
    preds = nc.dram_tensor(
        "preds", [BLOC, C, P, TPB * W], fp32, kind="ExternalInput"
    ).ap()
    if label_words == 1:
        labs3 = nc.dram_tensor(
            "labs", [BLOC, P, TPB * W], i32, kind="ExternalInput"
        ).ap()

        def lab_ap(b, lo, hi, plo=0, phi=P):
            return labs3[b, plo:phi, lo:hi]
    else:
        labs4 = nc.dram_tensor(
            "labs", [BLOC, P, TPB * W, label_words], i32, kind="ExternalInput"
        ).ap()

        def lab_ap(b, lo, hi, plo=0, phi=P):
            return labs4[b, plo:phi, lo:hi, 0:1]
    useam = nc.dram_tensor("useam", [BLOC, P, W], i32, kind="ExternalInput").ap()
    dseam = nc.dram_tensor("dseam", [BLOC, P, W], i32, kind="ExternalInput").ap()
    outp = nc.dram_tensor("partials", [1, 1024], fp32, kind="ExternalOutput").ap()

    with tile.TileContext(nc) as tc:
        with (
            tc.tile_pool(name="ps", bufs=1, space="PSUM") as ps_pool,
            tc.tile_pool(name="lab", bufs=2) as lab_pool,
            tc.tile_pool(name="xin", bufs=2) as x_pool,
            tc.tile_pool(name="eact", bufs=2) as e_pool,
            tc.tile_pool(name="sls", bufs=2) as s_pool,
            tc.tile_pool(name="wrk", bufs=1) as wrk,
            tc.tile_pool(name="xch", bufs=2) as xch_pool,
            tc.tile_pool(name="accp", bufs=1) as accp,
        ):
            ones = accp.tile([P, 1], bf16, name="ones")
            nc.vector.memset(ones[:], 1.0)
            negone = accp.tile([P, 1], fp32, name="negone")
            nc.vector.memset(negone[:], -1.0)
            pb = ps_pool.tile([1, 512], fp32, name="pb")
            pcb = ps_pool.tile([1, 512], fp32, name="pcb")
            SLABS = [(0, 512), (512, 1024), (1024, 1536), (1536, 2048), (2048, 2304)]
            for b in range(BLOC):
                # Us[p] = image row 6p-1 (clamped at top), Ds[p] = row
                # 6p+6 (clamped at bottom) -- prebuilt halo tensors so every
                # label DMA is a clean partition-aligned affine read.  One
                # tile per producer DMA so readers wait on exactly the
                # transfer they need.
                Lm = []
                Lmi = []

                def load_chunk_labels(h, b=b):
                    lmi = lab_pool.tile(
                        [P, RPC, W], i32, name=f"Lmi{h}", tag="Lmi"
                    )
                    nc.sync.dma_start(
                        out=lmi[:],
                        in_=lab_ap(b, h * RPC * W, (h + 1) * RPC * W),
                    )
                    Lmi.append(lmi)
                    lmc = lab_pool.tile(
                        [P, RPC, W], bf16, name=f"Lm{h}", tag=f"Lm{h}"
                    )
                    nc.scalar.activation(lmc[:], lmi[:], Act.Copy)
                    Lm.append(lmc)

                if b == 0:
                    # Warmup: interleave image-0's label loads with the first
                    # predictions chunk so neither engine waits on the whole
                    # label block before compute can start.  The x DMAs and
                    # seams are emitted inside the chunk loop below.
                    load_chunk_labels(0)
                    Us = lab_pool.tile([P, W], i32, name="Us", tag="Us")
                    Ds = lab_pool.tile([P, W], i32, name="Ds", tag="Ds")
                else:
                    Us = lab_pool.tile([P, W], i32, name="Us", tag="Us")
                    nc.sync.dma_start(out=Us[:], in_=useam[b, :, :])
                    Ds = lab_pool.tile([P, W], i32, name="Ds", tag="Ds")
                    nc.sync.dma_start(out=Ds[:], in_=dseam[b, :, :])
                    load_chunk_labels(0)
                    load_chunk_labels(1)
                for h in range(NH):
                    Lr = Lm[h][:]  # chunk label rows [P, RPC, W]
                    xs = []
                    for ch in range(C):
                        x = x_pool.tile([P, CHW], fp32, name=f"x{ch}", tag=f"x{ch}")
                        nc.sync.dma_start(
                            out=x[:],
                            in_=preds[b, ch, :, h * CHW : (h + 1) * CHW],
                        )
                        xs.append(x[:])
                    if b == 0 and h == 0:
                        nc.sync.dma_start(out=Us[:], in_=useam[b, :, :])
                        nc.sync.dma_start(out=Ds[:], in_=dseam[b, :, :])
                        load_chunk_labels(1)
                    # --- logsumexp numerator + x[label] selection -----------
                    # ACT emission order matters (in-order stream): produce
                    # xsel (needs only x0) and m2 (needs only labels) right
                    # after exp0 so the vector engine's CP chain is not
                    # blocked behind exp1/exp2/ln.
                    es = []
                    e = e_pool.tile([P, CHW], bf16, name="e0", tag="e0")
                    nc.scalar.activation(e[:], xs[0], Act.Exp)
                    es.append(e[:])
                    xsel = s_pool.tile([P, CHW], bf16, name="xsel", tag="xsel")
                    nc.scalar.activation(xsel[:], xs[0], Act.Copy)
                    m2 = s_pool.tile([P, RPC, W], bf16, name="m2", tag="m2")
                    nc.scalar.activation(m2[:], Lmi[h][:], Act.Relu, bias=negone[:])
                    for ch in range(1, C):
                        e = e_pool.tile([P, CHW], bf16, name=f"e{ch}", tag=f"e{ch}")
                        nc.scalar.activation(e[:], xs[ch], Act.Exp)
                        es.append(e[:])
                    # CP1 overwrites where L != 0 (the i32 labels are their
                    # own nonzero predicate), CP2 fixes up L == 2 with
                    # m2 = Relu(L - 1).  Ordered overwrites make the pair
                    # exact.  (CopyPredicated runs at 1x rate regardless of
                    # dtype, so selecting exps + a second Ln measured slower.)
                    # The CPs lead the vector stream: their inputs (xsel, m2)
                    # are ready before e1/e2, and they are the last readers
                    # of the x tiles, so finishing them early lets the next
                    # chunk's predictions DMAs start sooner.
                    nc.vector.copy_predicated(xsel[:], Lmi[h][:], xs[1])
                    i_cp2 = nc.vector.copy_predicated(
                        xsel[:], m2[:].bitcast(mybir.dt.int16), xs[2]
                    )
                    s1 = wrk.tile([P, CHW], bf16, name="s1", tag="s1")
                    nc.vector.tensor_add(s1[:], es[0], es[1])
                    s2 = s_pool.tile([P, CHW], bf16, name="s2", tag="s2")
                    nc.vector.tensor_add(s2[:], s1[:], es[2])
                    lse = s_pool.tile([P, CHW], bf16, name="lse", tag="lse")
                    nc.scalar.activation(lse[:], s2[:], Act.Ln)

                    # --- boundary mask --------------------------------------
                    nx = wrk.tile([P, RPC, W], bf16, name="nx", tag="nx")
                    nc.vector.tensor_tensor(
                        nx[:, :, 1 : W - 1],
                        Lr[:, :, 0 : W - 2],
                        Lr[:, :, 2:W],
                        Alu.not_equal,
                    )
                    nc.vector.tensor_tensor(
                        nx[:, :, 0:1], Lr[:, :, 0:1], Lr[:, :, 1:2], Alu.not_equal
                    )
                    nc.vector.tensor_tensor(
                        nx[:, :, W - 1 : W],
                        Lr[:, :, W - 2 : W - 1],
                        Lr[:, :, W - 1 : W],
                        Alu.not_equal,
                    )
                    # ny row r compares image rows r-1 and r+1; rows live in
                    # (Us | Lm0 | Lm1 | Ds) tiles, so emit one inst per row
                    # with exactly the producers it needs.
                    ny = wrk.tile([P, RPC, W], bf16, name="ny", tag="ny")
                    if h == 0:
                        pairs = [
                            (Us[:], Lm[0][:, 1, :]),
                            (Lm[0][:, 0, :], Lm[0][:, 2, :]),
                            (Lm[0][:, 1, :], Lm[1][:, 0, :]),
                        ]
                    else:
                        pairs = [
                            (Lm[0][:, 2, :], Lm[1][:, 1, :]),
                            (Lm[1][:, 0, :], Lm[1][:, 2, :]),
                            (Lm[1][:, 1, :], Ds[:]),
                        ]
                    # Order-only edges: keep the late-arriving-label ny rows
                    # behind the predictions-dependent chain so they cannot
                    # head-block the in-order vector stream during warmup.
                    for j, (top, bot) in enumerate(pairs):
                        i_ny = nc.vector.tensor_tensor(
                            ny[:, j, :], top, bot, Alu.not_equal
                        )
                        _add_dep(i_ny.ins, i_cp2.ins, sync=False,
                                 reason="schedule ny after CP chain")
                    bnd = wrk.tile([P, CHW], bf16, name="bnd", tag="bnd")
                    nc.vector.tensor_tensor(bnd[:], nx[:], ny[:], Alu.max)

                    # --- weighted CE and reductions -------------------------
                    ce = wrk.tile([P, CHW], bf16, name="ce", tag="ce")
                    nc.vector.tensor_sub(ce[:], lse[:], xsel[:])
                    cb = wrk.tile([P, CHW], bf16, name="cb", tag="cb")
                    nc.vector.tensor_mul(cb[:], ce[:], bnd[:])

                    first = b == 0 and h == 0
                    last = b == BLOC - 1 and h == NH - 1
                    for k, (a0, a1) in enumerate(SLABS):
                        nc.tensor.matmul(
                            pb[:, 0 : a1 - a0],
                            ones[:],
                            bnd[:, a0:a1],
                            start=first and k == 0,
                            stop=last and k == len(SLABS) - 1,
                        )
                        nc.tensor.matmul(
                            pcb[:, 0 : a1 - a0],
                            ones[:],
                            cb[:, a0:a1],
                            start=first and k == 0,
                            stop=last and k == len(SLABS) - 1,
                        )
            sb = wrk.tile([1, 1024], fp32, name="sb")
            nc.vector.tensor_copy(sb[:, 0:512], pb[:, :])
            nc.vector.tensor_copy(sb[:, 512:1024], pcb[:, :])
            nc.sync.dma_start(out=outp[:, :], in_=sb[:])

    # Pin Exp/Ln/Copy/Relu to the one table set containing all of them so the
    # ACT table loads once instead of thrashing between sets.
    from concourse import hw_specs

    KEEP = "natural_log_exp_and_others"
    orig = hw_specs.get_activation_tables

    def only_combined(arch):
        t = orig(arch)
        return {name: (funcs if name == KEEP else set()) for name, funcs in t.items()}

    patched = []
    for mod in (hw_specs, bacc):
        if getattr(mod, "get_activation_tables", None) is not None:
            patched.append((mod, mod.get_activation_tables))
            mod.get_activation_tables = only_combined
    try:
        nc.compile()
    finally:
        for mod, fn in patched:
            mod.get_activation_tables = fn
    return nc


def _get_nc(label_words):
    if label_words not in _CACHE:
        _CACHE[label_words] = _build(label_words)
    return _CACHE[label_words]


def kernel(predictions, labels):
    from concourse.bass_utils import run_bass_kernel_spmd

    preds = np.ascontiguousarray(predictions, dtype=np.float32).reshape(
        NCORES, BLOC, C, P, TPB * W
    )
    labels = np.ascontiguousarray(labels)
    if labels.dtype == np.int64:
        label_words = 2
        labs32 = labels.view("<i4").reshape(NCORES, BLOC, P, TPB * W, 2)
    elif labels.dtype == np.int32:
        label_words = 1
        labs32 = labels.reshape(NCORES, BLOC, P, TPB * W)
    else:
        raise ValueError(f"unsupported labels dtype {labels.dtype}")

    # Halo rows for the vertical-gradient seams: useam[b, p] = image row
    # 6p-1 (clamped at the top), dseam[b, p] = row 6p+6 (clamped at the
    # bottom).  Pure gather/layout, so it stays host-side sharding prep.
    lab_rows = labels.reshape(NCORES, BLOC, H, W)
    uidx = np.maximum(TPB * np.arange(P) - 1, 0)
    didx = np.minimum(TPB * np.arange(P) + TPB, H - 1)
    useam = np.ascontiguousarray(lab_rows[:, :, uidx, :], dtype=np.int32)
    dseam = np.ascontiguousarray(lab_rows[:, :, didx, :], dtype=np.int32)

    nc = _get_nc(label_words)
    in_maps = [
        {"preds": preds[i], "labs": labs32[i], "useam": useam[i],
         "dseam": dseam[i]}
        for i in range(NCORES)
    ]
    res = run_bass_kernel_spmd(nc, in_maps, list(range(NCORES))).results
    tot_b = 0.0
    tot_cb = 0.0
    for r in res:
        p = r["partials"].astype(np.float64)
        tot_b += p[0, :512].sum()
        tot_cb += p[0, 512:].sum()
    return np.float32(tot_cb / (tot_b + 1e-8))


# revision 48
# speedup vs baseline: 1.0381x; 1.0381x over previous
"""BoundaryLoss kernel for 8 Trainium2 NeuronCores.

Math (equivalent to the reference):
  boundary(i,j) = [L(i,j+1) != L(i,j-1)]_edge OR [L(i+1,j) != L(i-1,j)]_edge
    (the union of class-1/class-2 indicator boundaries equals "any label
     change" because any differing pair in {0,1,2} differs in membership
     of class 1 or class 2; |gx|+|gy| > 0.1 iff either diff is nonzero)
  ce = logsumexp_c(x) - x[label]        (max-free: |x| <= ~6 so exp is safe)
  loss = sum(ce * boundary) / (sum(boundary) + 1e-8)

Sharding: pure data parallel, 4 images per core.  Each core writes
per-column partial sums of (boundary, ce*boundary); the host sums the
8 * [1, 1024] partials in float64 and does the final division.

Design notes (what made this fast; measured on HW traces):
  - ALL loads ride the pipelined HWDGE queue as plain dtypes.  The SWDGE
    (gpsimd) cast-DMA queue is serial (~6.5us per dispatch) and GpSimd
    *compute* locks the vector engine out of its 2-port perf modes
    (measured 2-3x slowdowns on concurrently-running DVE ops), so
    neither is used.  Label i32 -> bf16 casts run on the scalar engine.
  - Seam rows (partition-crossing row neighbours for the vertical
    gradient) come from small host-prepared halo tensors; partition-
    shifted on-device copies degrade to per-partition descriptors that
    pile up on one DMA engine.
  - x[label] is selected with copy_predicated overwrites (x0, then x1
    where L != 0 using the label bits as the predicate, then x2 where
    Relu(L-1) != 0) instead of exp-domain masked products + a second
    Ln: one activation pass and several DVE passes cheaper.
  - per-pixel boundary / ce*boundary sums reduce on the tensor engine
    (matmul with a ones vector), which is otherwise idle; work is
    balanced so scalar ~= vector ~= DMA ~= 120us per core.
"""

import numpy as np

B, C, H, W = 32, 3, 768, 768
NCORES = 8
BLOC = B // NCORES  # images per core
P = 128
TPB = H // P        # rows per partition (6)
NH = 2              # chunks (halves) per image
RPC = TPB // NH     # rows per chunk (3)
CHW = RPC * W       # columns per chunk (2304)

_CACHE = {}


def _build(label_words):
    """Build + compile the Bass module. label_words = int32 words per label
    element (2 for int64 inputs, 1 for int32)."""
    import concourse.bacc as bacc
    import concourse.tile as tile
    import concourse.mybir as mybir
    from bass_rust import add_dep_helper as _add_dep

    fp32 = mybir.dt.float32
    bf16 = mybir.dt.bfloat16
    i32 = mybir.dt.int32
    Alu = mybir.AluOpType
    Act = mybir.ActivationFunctionType

    nc = bacc.Bacc(
        "TRN2",
        target_bir_lowering=False,
        debug=False,
        enable_asserts=False,
        num_devices=NCORES,
    )
    preds = nc.dram_tensor(
        "preds", [BLOC, C, P, TPB * W], fp32, kind="ExternalInput"
    ).ap()
    if label_words == 1:
        labs3 = nc.dram_tensor(
            "labs", [BLOC, P, TPB * W], i32, kind="ExternalInput"
        ).ap()

        def lab_ap(b, lo, hi, plo=0, phi=P):
            return labs3[b, plo:phi, lo:hi]
    else:
        labs4 = nc.dram_tensor(
            "labs", [BLOC, P, TPB * W, label_words], i32, kind="ExternalInput"
        ).ap()

        def lab_ap(b, lo, hi, plo=0, phi=P):
            return labs4[b, plo:phi, lo:hi, 0:1]
    useam = nc.dram_tensor("useam", [BLOC, P, W], i32, kind="ExternalInput").ap()
    dseam = nc.dram_tensor("dseam", [BLOC, P, W], i32, kind="ExternalInput").ap()
    outp = nc.dram_tensor("partials", [1, 1024], fp32, kind="ExternalOutput").ap()

    with tile.TileContext(nc) as tc:
        with (
            tc.tile_pool(name="ps", bufs=1, space="PSUM") as ps_pool,
            tc.tile_pool(name="lab", bufs=2) as lab_pool,
            tc.tile_pool(name="xin", bufs=2) as x_pool,
            tc.tile_pool(name="eact", bufs=2) as e_pool,
            tc.tile_pool(name="sls", bufs=2) as s_pool,
            tc.tile_pool(name="wrk", bufs=1) as wrk,
            tc.tile_pool(name="xch", bufs=2) as xch_pool,
            tc.tile_pool(name="accp", bufs=1) as accp,
        ):
            ones = accp.tile([P, 1], bf16, name="ones")
            nc.vector.memset(ones[:], 1.0)
            negone = accp.tile([P, 1], fp32, name="negone")
            nc.vector.memset(negone[:], -1.0)
            pb = ps_pool.tile([1, 512], fp32, name="pb")
            pcb = ps_pool.tile([1, 512], fp32, name="pcb")
            SLABS = [(0, 512), (512, 1024), (1024, 1536), (1536, 2048), (2048, 2304)]
            for b in range(BLOC):
                # Us[p] = image row 6p-1 (clamped at top), Ds[p] = row
                # 6p+6 (clamped at bottom) -- prebuilt halo tensors so every
                # label DMA is a clean partition-aligned affine read.  One
                # tile per producer DMA so readers wait on exactly the
                # transfer they need.
                Lm = []
                Lmi = []

                def load_chunk_labels(h, b=b):
                    lmi = lab_pool.tile(
                        [P, RPC, W], i32, name=f"Lmi{h}", tag="Lmi"
                    )
                    nc.sync.dma_start(
                        out=lmi[:],
                        in_=lab_ap(b, h * RPC * W, (h + 1) * RPC * W),
                    )
                    Lmi.append(lmi)
                    lmc = lab_pool.tile(
                        [P, RPC, W], bf16, name=f"Lm{h}", tag=f"Lm{h}"
                    )
                    nc.scalar.activation(lmc[:], lmi[:], Act.Copy)
                    Lm.append(lmc)

                # Interleave each image's label loads with its first
                # predictions chunk: chunk-0 labels load first, then the
                # chunk-0 x DMAs (emitted in the loop below), then the seams
                # and chunk-1 labels.  This keeps the CP chain's x inputs
                # from queueing behind a 3 MB label block at every image
                # boundary.
                load_chunk_labels(0)
                Us = lab_pool.tile([P, W], i32, name="Us", tag="Us")
                Ds = lab_pool.tile([P, W], i32, name="Ds", tag="Ds")
                for h in range(NH):
                    Lr = Lm[h][:]  # chunk label rows [P, RPC, W]
                    xs = []
                    for ch in range(C):
                        x = x_pool.tile([P, CHW], fp32, name=f"x{ch}", tag=f"x{ch}")
                        nc.sync.dma_start(
                            out=x[:],
                            in_=preds[b, ch, :, h * CHW : (h + 1) * CHW],
                        )
                        xs.append(x[:])
                    if h == 0:
                        nc.sync.dma_start(out=Us[:], in_=useam[b, :, :])
                        nc.sync.dma_start(out=Ds[:], in_=dseam[b, :, :])
                        load_chunk_labels(1)
                    # --- logsumexp numerator + x[label] selection -----------
                    # ACT emission order matters (in-order stream): produce
                    # xsel (needs only x0) and m2 (needs only labels) right
                    # after exp0 so the vector engine's CP chain is not
                    # blocked behind exp1/exp2/ln.
                    es = []
                    e = e_pool.tile([P, CHW], bf16, name="e0", tag="e0")
                    nc.scalar.activation(e[:], xs[0], Act.Exp)
                    es.append(e[:])
                    xsel = s_pool.tile([P, CHW], bf16, name="xsel", tag="xsel")
                    nc.scalar.activation(xsel[:], xs[0], Act.Copy)
                    m2 = s_pool.tile([P, RPC, W], bf16, name="m2", tag="m2")
                    nc.scalar.activation(m2[:], Lmi[h][:], Act.Relu, bias=negone[:])
                    for ch in range(1, C):
                        e = e_pool.tile([P, CHW], bf16, name=f"e{ch}", tag=f"e{ch}")
                        nc.scalar.activation(e[:], xs[ch], Act.Exp)
                        es.append(e[:])
                    # CP1 overwrites where L != 0 (the i32 labels are their
                    # own nonzero predicate), CP2 fixes up L == 2 with
                    # m2 = Relu(L - 1).  Ordered overwrites make the pair
                    # exact.  (CopyPredicated runs at 1x rate regardless of
                    # dtype, so selecting exps + a second Ln measured slower.)
                    # The CPs lead the vector stream: their inputs (xsel, m2)
                    # are ready before e1/e2, and they are the last readers
                    # of the x tiles, so finishing them early lets the next
                    # chunk's predictions DMAs start sooner.
                    nc.vector.copy_predicated(xsel[:], Lmi[h][:], xs[1])
                    i_cp2 = nc.vector.copy_predicated(
                        xsel[:], m2[:].bitcast(mybir.dt.int16), xs[2]
                    )
                    s1 = wrk.tile([P, CHW], bf16, name="s1", tag="s1")
                    nc.vector.tensor_add(s1[:], es[0], es[1])
                    s2 = s_pool.tile([P, CHW], bf16, name="s2", tag="s2")
                    nc.vector.tensor_add(s2[:], s1[:], es[2])
                    lse = s_pool.tile([P, CHW], bf16, name="lse", tag="lse")
                    nc.scalar.activation(lse[:], s2[:], Act.Ln)

                    # --- boundary mask --------------------------------------
                    nx = wrk.tile([P, RPC, W], bf16, name="nx", tag="nx")
                    nc.vector.tensor_tensor(
                        nx[:, :, 1 : W - 1],
                        Lr[:, :, 0 : W - 2],
                        Lr[:, :, 2:W],
                        Alu.not_equal,
                    )
                    nc.vector.tensor_tensor(
                        nx[:, :, 0:1], Lr[:, :, 0:1], Lr[:, :, 1:2], Alu.not_equal
                    )
                    nc.vector.tensor_tensor(
                        nx[:, :, W - 1 : W],
                        Lr[:, :, W - 2 : W - 1],
                        Lr[:, :, W - 1 : W],
                        Alu.not_equal,
                    )
                    # ny row r compares image rows r-1 and r+1; rows live in
                    # (Us | Lm0 | Lm1 | Ds) tiles, so emit one inst per row
                    # with exactly the producers it needs.
                    ny = wrk.tile([P, RPC, W], bf16, name="ny", tag="ny")
                    if h == 0:
                        pairs = [
                            (Us[:], Lm[0][:, 1, :]),
                            (Lm[0][:, 0, :], Lm[0][:, 2, :]),
                            (Lm[0][:, 1, :], Lm[1][:, 0, :]),
                        ]
                    else:
                        pairs = [
                            (Lm[0][:, 2, :], Lm[1][:, 1, :]),
                            (Lm[1][:, 0, :], Lm[1][:, 2, :]),
                            (Lm[1][:, 1, :], Ds[:]),
                        ]
                    # Order-only edges: keep the late-arriving-label ny rows
                    # behind the predictions-dependent chain so they cannot
                    # head-block the in-order vector stream during warmup.
                    for j, (top, bot) in enumerate(pairs):
                        i_ny = nc.vector.tensor_tensor(
                            ny[:, j, :], top, bot, Alu.not_equal
                        )
                        _add_dep(i_ny.ins, i_cp2.ins, sync=False,
                                 reason="schedule ny after CP chain")
                    bnd = wrk.tile([P, CHW], bf16, name="bnd", tag="bnd")
                    nc.vector.tensor_tensor(bnd[:], nx[:], ny[:], Alu.max)

                    # --- weighted CE and reductions -------------------------
                    ce = wrk.tile([P, CHW], bf16, name="ce", tag="ce")
                    nc.vector.tensor_sub(ce[:], lse[:], xsel[:])
                    cb = wrk.tile([P, CHW], bf16, name="cb", tag="cb")
                    nc.vector.tensor_mul(cb[:], ce[:], bnd[:])

                    first = b == 0 and h == 0
                    last = b == BLOC - 1 and h == NH - 1
                    for k, (a0, a1) in enumerate(SLABS):
                        nc.tensor.matmul(
                            pb[:, 0 : a1 - a0],
                            ones[:],
                            bnd[:, a0:a1],
                            start=first and k == 0,
                            stop=last and k == len(SLABS) - 1,
                        )
                        nc.tensor.matmul(
                            pcb[:, 0 : a1 - a0],
                            ones[:],
                            cb[:, a0:a1],
                            start=first and k == 0,
                            stop=last and k == len(SLABS) - 1,
                        )
            sb = wrk.tile([1, 1024], fp32, name="sb")
            nc.vector.tensor_copy(sb[:, 0:512], pb[:, :])
            nc.vector.tensor_copy(sb[:, 512:1024], pcb[:, :])
            nc.sync.dma_start(out=outp[:, :], in_=sb[:])

    # Pin Exp/Ln/Copy/Relu to the one table set containing all of them so the
    # ACT table loads once instead of thrashing between sets.
    from concourse import hw_specs

    KEEP = "natural_log_exp_and_others"
    orig = hw_specs.get_activation_tables

    def only_combined(arch):
        t = orig(arch)
        return {name: (funcs if name == KEEP else set()) for name, funcs in t.items()}

    patched = []
    for mod in (hw_specs, bacc):
        if getattr(mod, "get_activation_tables", None) is not None:
            patched.append((mod, mod.get_activation_tables))
            mod.get_activation_tables = only_combined
    try:
        nc.compile()
    finally:
        for mod, fn in patched:
            mod.get_activation_tables = fn
    return nc


def _get_nc(label_words):
    if label_words not in _CACHE:
        _CACHE[label_words] = _build(label_words)
    return _CACHE[label_words]


def kernel(predictions, labels):
    from concourse.bass_utils import run_bass_kernel_spmd

    preds = np.ascontiguousarray(predictions, dtype=np.float32).reshape(
        NCORES, BLOC, C, P, TPB * W
    )
    labels = np.ascontiguousarray(labels)
    if labels.dtype == np.int64:
        label_words = 2
        labs32 = labels.view("<i4").reshape(NCORES, BLOC, P, TPB * W, 2)
    elif labels.dtype == np.int32:
        label_words = 1
        labs32 = labels.reshape(NCORES, BLOC, P, TPB * W)
    else:
        raise ValueError(f"unsupported labels dtype {labels.dtype}")

    # Halo rows for the vertical-gradient seams: useam[b, p] = image row
    # 6p-1 (clamped at the top), dseam[b, p] = row 6p+6 (clamped at the
    # bottom).  Pure gather/layout, so it stays host-side sharding prep.
    lab_rows = labels.reshape(NCORES, BLOC, H, W)
    uidx = np.maximum(TPB * np.arange(P) - 1, 0)
    didx = np.minimum(TPB * np.arange(P) + TPB, H - 1)
    useam = np.ascontiguousarray(lab_rows[:, :, uidx, :], dtype=np.int32)
    dseam = np.ascontiguousarray(lab_rows[:, :, didx, :], dtype=np.int32)

    nc = _get_nc(label_words)
    in_maps = [
        {"preds": preds[i], "labs": labs32[i], "useam": useam[i],
         "dseam": dseam[i]}
        for i in range(NCORES)
    ]
    res = run_bass_kernel_spmd(nc, in_maps, list(range(NCORES))).results
    tot_b = 0.0
    tot_cb = 0.0
    for r in res:
        p = r["partials"].astype(np.float64)
        tot_b += p[0, :512].sum()
        tot_cb += p[0, 512:].sum()
    return np.float32(tot_cb / (tot_b + 1e-8))
